# revision 19
# baseline (speedup 1.0000x reference)
"""MDTA (channel-attention transformer block) Trainium2 kernel, v2.

Math (validated against the jax reference):
  xn = LayerNorm(x) = z * gamma + beta,  z = (x - mu) * rsqrt(var + eps)
  Q/K/V = xn @ W* + b*;  scores_h = K_h^T Q_h / alpha  (per-head s x s, contracted
  over all t tokens);  attn = softmax(scores);  out = V @ blockdiag(attn)
  y = out @ Wf + bf + xn

With zero biases/beta (the spec fill), everything collapses to:
  G      = z^T z                        (C x C Gram, contracted over t)
  scores = Wk'^T G Wq' / alpha          (Wq' = diag(gamma) Wq, etc.)
  attn   = blockwise softmax(scores)    (4 diagonal 32x32 blocks)
  W2     = diag(g) Wv blockdiag(attn) Wf + diag(gamma)
  y      = z @ W2

Scaling trick: the device works with z' = z/sqrt(C) (rstd' = 1/sqrt(M2 + C*eps)
skips the /C in the variance). The C factor is restored in the score extraction
(x C) and by pre-scaling Wf and diag(gamma) by sqrt(C) on the host.

Sharding: 8 cores = (batch b in 0..3) x (token half in 0..1). Each core streams
its 32768-token slice in fp16: bn_stats (DVE) + even/odd merge -> mu, rstd';
z' = (x-mu)*rstd' split DVE(4x-mode)/ACT; Gram accumulation + transpose of z'
on PE with GPSIMD draining PSUM->SBUF. The last quarter of transposes is
deferred to overlap the pairwise AllReduce of G. Softmax/W2 are computed
redundantly per pair, then y'^T = W2'^T z'^T streams out in fp16 chunks.

v1 (previous session) measured 708us on HW: the per-element normalization sat
on GPSIMD at 2.26us/tile (7 G elem/s) and serialized the whole kernel. v2
moves it to DVE's 4x fp16 mode / ACT, halves DMA via fp16 I/O, drops bn_aggr
for a batched even/odd merge, and keeps PE near its ramped clock.
"""

import sys

import numpy as np

for _p in ("/opt/trn_rl_repo",):
    if _p not in sys.path:
        sys.path.append(_p)

import concourse.bacc as bacc
import concourse.bass as bass
import concourse.tile as tile
from concourse import mybir
from concourse.bass_utils import run_bass_kernel_spmd

B, HH, WW, C = 4, 256, 256, 128
NH, S = 4, 32
T = HH * WW            # tokens per batch
N_CORES = 8
TLOC = T // 2          # tokens per core
EPS = 1e-5
P = 128                # partitions / tile token count
LGRP = 8               # tiles per x-load DMA (2KB partition lines)
XB = 8                 # tiles per XBAR transpose instruction
SGRP = 4               # tiles per bn_stats call (free dim 512 = fmax)
MB = 8                 # stats-groups per merge batch (32 tiles)
XPAD = 0               # no pad: stats are per-tile accumulate ops
CP = C + XPAD
YCHUNK = SGRP * P      # output-stream chunk (1 PSUM bank)

F32 = mybir.dt.float32
F16 = mybir.dt.float16

AOP = mybir.AluOpType
AF = mybir.ActivationFunctionType


def build_nc(tloc=TLOC, n_cores=N_CORES, inv_alpha=1.0, zdt=F16, y_f32r=False,
             act_groups=5, defer_frac=0.25, y_psum_f16=True):
    """Build the SPMD Bass program. Cores 2b and 2b+1 hold the two token
    halves of batch b and pair up in the G all-reduce.

    act_groups: out of every 8 stats-groups, how many are normalized on ACT
    (the rest on DVE). defer_frac: fraction of transpose groups deferred to
    overlap the collective."""
    ntile = tloc // P
    nlg = ntile // LGRP
    nsg = ntile // SGRP
    mbs = min(MB, nsg)
    assert nsg % mbs == 0 and ntile % LGRP == 0
    nmb = nsg // mbs
    nychunk = tloc // YCHUNK            # == nsg
    dsg = max(1, int(round(nsg * (1.0 - defer_frac))))  # first deferred group
    sqC = float(np.sqrt(C))

    nc = bacc.Bacc("TRN2", target_bir_lowering=False, debug=False,
                   num_devices=n_cores)

    x_in = nc.declare_dram_parameter("x_loc", [nlg, P, LGRP * CP], F16,
                                     isOutput=False)   # host-repacked fp16
    wq_in = nc.declare_dram_parameter("wq_g", [C, C], F32, isOutput=False)
    wk_in = nc.declare_dram_parameter("wk_g", [C, C], F32, isOutput=False)
    wvT_in = nc.declare_dram_parameter("wvT4", [S, NH * C], F32, isOutput=False)
    wf_in = nc.declare_dram_parameter("wf_s", [C, C], F32, isOutput=False)   # sqrt(C)*Wf
    dg_in = nc.declare_dram_parameter("dg_s", [C, C], F32, isOutput=False)   # sqrt(C)*diag(gamma)
    idz_in = nc.declare_dram_parameter("ident_z", [P, P], F16, isOutput=False)
    id32_in = nc.declare_dram_parameter("ident_f32", [P, P], F32, isOutput=False)
    yT_out = nc.declare_dram_parameter("yT", [C, tloc], F16, isOutput=True)

    x_tiles = x_in.rearrange("g p (j c) -> g p j c", j=LGRP)  # c = CP
    replica_groups = [[2 * b, 2 * b + 1] for b in range(n_cores // 2)]

    # per-tile normalization engines, by measured rates
    # (DVE tensor_scalar 270ns, ACT activation 330ns, GPSIMD 2.26us)
    norm_eng = "aavaagaaaaagavaaaaaagaaavaaaaava"  # 25a 4v 3g per 32 tiles

    with tile.TileContext(nc) as tc:
        with (
            tc.tile_pool(name="const", bufs=1) as const,
            tc.tile_pool(name="xload", bufs=10) as xload,
            tc.tile_pool(name="tmp", bufs=3) as tmp,
            tc.tile_pool(name="small", bufs=2) as small,
            tc.tile_pool(name="ybuf", bufs=4) as ybuf,
            tc.tile_pool(name="psA", bufs=1, space="PSUM") as psA,
            tc.tile_pool(name="psS", bufs=2, space="PSUM") as psS,
            tc.tile_pool(name="psY", bufs=4, space="PSUM") as psY,
            tc.tile_pool(name="dram", bufs=1, space="DRAM") as dram,
        ):
            # ---- constants ----
            wq_sb = const.tile([C, C], F32)
            wk_sb = const.tile([C, C], F32)
            wvT_sb = const.tile([S, NH, C], F32)
            wf_sb = const.tile([C, C], F32)
            dg_sb = const.tile([C, C], F32)
            idz_sb = const.tile([P, P], F16)
            id32_sb = const.tile([P, P], F32)
            nc.sync.dma_start(out=wq_sb, in_=wq_in[:])
            nc.sync.dma_start(out=wk_sb, in_=wk_in[:])
            nc.sync.dma_start(out=wvT_sb,
                              in_=wvT_in[:].rearrange("s (h c) -> s h c", h=NH))
            nc.sync.dma_start(out=wf_sb, in_=wf_in[:])
            nc.sync.dma_start(out=dg_sb, in_=dg_in[:])
            nc.sync.dma_start(out=idz_sb, in_=idz_in[:])
            nc.sync.dma_start(out=id32_sb, in_=id32_in[:])
            eps_sb = const.tile([P, 1], F32)
            nc.vector.memset(eps_sb, float(C) * EPS)

            # ---- big write-once state ----
            zbig = const.tile([P, ntile, C], F16)     # z' in [t, c]
            zT = const.tile([C, ntile, P], F16)       # z' transposed (flat [C, tloc])
            st6 = const.tile([P, nsg, SGRP, 8], F32)  # bn_stats out (6 used)
            mub = const.tile([P, nsg, SGRP], F32)     # mean
            stdb = const.tile([P, nsg, SGRP], F32)    # std' (ACT sqrt out)
            rsb = const.tile([P, nsg, SGRP], F32)     # rstd' = 1/sqrt(M2+C eps)
            nmb_t = const.tile([P, nsg, SGRP], F32)   # -mean*rstd' (ACT bias)
            G_ps = psA.tile([C, C], F32)              # Gram accumulator

            # ============ Phase 1: stats + norm + Gram (+ some T) ==========
            xh = [None] * nlg

            def do_xbar(xg, eng):
                """Transpose z' tiles [16*xg, 16*xg+16) via the DMA XBAR
                (SBUF->SBUF, per-128x128-tile): zT[c, i, t] = z[t, i, c]."""
                src_ap = zbig[:, XB * xg:XB * (xg + 1), :]
                eng.dma_start(out=zT[:, XB * xg:XB * (xg + 1)], in_=src_ap,
                              transpose=True)

            def load_and_stats(mb):
                nlg_b = mbs * SGRP // LGRP
                for lgi in range(nlg_b):
                    lg = mb * nlg_b + lgi
                    x8 = xload.tile([P, LGRP, CP], F16)
                    xh[lg] = x8
                    nc.sync.dma_start(out=x8, in_=x_tiles[lg])
                    for jl in range(LGRP):
                        i = lg * LGRP + jl
                        sg, jt = i // SGRP, i % SGRP
                        nc.vector.bn_stats(out=st6[:, sg, jt, 0:6],
                                           in_=x8[:, jl, 0:C])

            def merge_a(mb):
                """DVE moment combine + ACT sqrt for batch mb (no recip)."""
                g0, g1 = mbs * mb, mbs * mb + mbs
                me = st6[:, g0:g1, :, 1]
                mo = st6[:, g0:g1, :, 4]
                ve = st6[:, g0:g1, :, 2]
                vo = st6[:, g0:g1, :, 5]
                t_s = tmp.tile([P, mbs, SGRP], F32)
                t_d = tmp.tile([P, mbs, SGRP], F32)
                t_dd = tmp.tile([P, mbs, SGRP], F32)
                t_ve = tmp.tile([P, mbs, SGRP], F32)
                t_m2 = tmp.tile([P, mbs, SGRP], F32)
                t_std = stdb[:, g0:g1, :]
                mu = mub[:, g0:g1, :]
                nc.vector.tensor_tensor(out=t_s, in0=me, in1=mo, op=AOP.add)
                nc.vector.tensor_scalar_mul(out=mu, in0=t_s, scalar1=0.5)
                nc.vector.tensor_tensor(out=t_d, in0=me, in1=mo, op=AOP.subtract)
                nc.vector.tensor_tensor(out=t_dd, in0=t_d, in1=t_d, op=AOP.mult)
                nc.vector.tensor_tensor(out=t_ve, in0=ve, in1=vo, op=AOP.add)
                nc.vector.scalar_tensor_tensor(
                    out=t_m2, in0=t_dd, scalar=float(C) / 4.0, in1=t_ve,
                    op0=AOP.mult, op1=AOP.add)
                # std' = sqrt(M2 + C*eps) on ACT; latency hidden by the next
                # batch's bn_stats on DVE
                nc.scalar.activation(out=t_std, in_=t_m2, func=AF.Sqrt,
                                     bias=eps_sb[:], scale=1.0)

            def merge_b(mb):
                """rstd' = 1/std' and -mu*rstd' for batch mb."""
                g0, g1 = mbs * mb, mbs * mb + mbs
                t_mr = tmp.tile([P, mbs, SGRP], F32)
                mu = mub[:, g0:g1, :]
                rs = rsb[:, g0:g1, :]
                nm = nmb_t[:, g0:g1, :]
                nc.vector.reciprocal(out=rs, in_=stdb[:, g0:g1, :])
                nc.vector.tensor_tensor(out=t_mr, in0=mu, in1=rs, op=AOP.mult)
                nc.vector.tensor_scalar_mul(out=nm, in0=t_mr, scalar1=-1.0)

            def norm_gram(mb):
                for sgi in range(mbs):
                    sg = mbs * mb + sgi
                    for jt in range(SGRP):
                        i = sg * SGRP + jt
                        lg, jl = i // LGRP, i % LGRP
                        xs = xh[lg][:, jl, 0:C]
                        zi = zbig[:, i, :]
                        eng = norm_eng[i % len(norm_eng)]
                        if eng == "a":
                            nc.scalar.activation(
                                out=zi, in_=xs, func=AF.Identity,
                                bias=nmb_t[:, sg, jt:jt + 1],
                                scale=rsb[:, sg, jt:jt + 1])
                        elif eng == "g":
                            nc.gpsimd.tensor_scalar(
                                out=zi, in0=xs,
                                scalar1=mub[:, sg, jt:jt + 1],
                                scalar2=rsb[:, sg, jt:jt + 1],
                                op0=AOP.subtract, op1=AOP.mult)
                        else:
                            nc.vector.tensor_scalar(
                                out=zi, in0=xs,
                                scalar1=mub[:, sg, jt:jt + 1],
                                scalar2=rsb[:, sg, jt:jt + 1],
                                op0=AOP.subtract, op1=AOP.mult)
                        nc.tensor.matmul(G_ps, lhsT=zi, rhs=zi,
                                         start=(i == 0), stop=(i == ntile - 1))

            load_and_stats(0)
            merge_a(0)
            for mb in range(nmb):
                if mb + 1 < nmb:
                    load_and_stats(mb + 1)
                    merge_a(mb + 1)
                merge_b(mb)
                norm_gram(mb)
                if mb > 0:
                    for xg in range(mbs * SGRP * (mb - 1) // XB,
                                    mbs * SGRP * mb // XB):
                        do_xbar(xg, nc.sync)

            # ============ Phase 2: all-reduce G, softmax, W2 ===============
            for xg in range(mbs * SGRP * (nmb - 1) // XB, ntile // XB):
                do_xbar(xg, nc.sync)
            g_sb = small.tile([C, C], F32)
            nc.vector.tensor_copy(out=g_sb, in_=G_ps)
            g_in_d = dram.tile([C, C], F32)
            g_out_d = dram.tile([C, C], F32)
            nc.gpsimd.dma_start(out=g_in_d, in_=g_sb)
            nc.gpsimd.collective_compute(
                "AllReduce", AOP.add,
                replica_groups=replica_groups,
                ins=[g_in_d[:].opt()], outs=[g_out_d[:].opt()])

            gs_sb = small.tile([C, C], F32)
            nc.gpsimd.dma_start(out=gs_sb, in_=g_out_d)

            # scores = wk^T (G wq) * (C/alpha);  G symmetric so lhsT=G works
            s1_ps = psS.tile([C, C], F32, tag="ph2")
            nc.tensor.matmul(s1_ps, lhsT=gs_sb, rhs=wq_sb, start=True, stop=True)
            s1_sb = small.tile([C, C], F32)
            nc.scalar.copy(out=s1_sb, in_=s1_ps)
            sc_ps = psS.tile([C, C], F32, tag="ph2")
            nc.tensor.matmul(sc_ps, lhsT=wk_sb, rhs=s1_sb, start=True, stop=True)

            # extract 4 diagonal 32x32 blocks (x C/alpha) -> [128, 32]
            sm = small.tile([P, S], F32)
            for h in range(NH):
                nc.scalar.mul(out=sm[h * S:(h + 1) * S, :],
                              in_=sc_ps[h * S:(h + 1) * S, h * S:(h + 1) * S],
                              mul=float(C) * float(inv_alpha))
            mx = small.tile([P, 1], F32)
            nc.vector.reduce_max(mx, sm, mybir.AxisListType.X)
            nmx = small.tile([P, 1], F32)
            nc.vector.tensor_scalar_mul(out=nmx, in0=mx, scalar1=-1.0)
            sh = small.tile([P, S], F32)
            nc.vector.tensor_scalar(out=sh, in0=sm, scalar1=nmx, scalar2=-87.0,
                                    op0=AOP.add, op1=AOP.max)
            ex = small.tile([P, S], F32)
            es = small.tile([P, 1], F32)
            nc.scalar.activation(out=ex, in_=sh, func=AF.Exp,
                                 bias=0.0, scale=1.0, accum_out=es)
            ri = small.tile([P, 1], F32)
            nc.vector.reciprocal(out=ri, in_=es)
            at = small.tile([P, S], F32)
            nc.vector.tensor_scalar_mul(out=at, in0=ex, scalar1=ri)
            at4 = small.tile([S, NH, S], F32)
            for h in range(NH):
                nc.sync.dma_start(out=at4[:, h, :], in_=at[h * S:(h + 1) * S, :])

            # U = diag(g) Wv blockdiag(attn): per-head [128,32] matmuls
            u_ps = psS.tile([C, C], F32, tag="ph2")
            for h in range(NH):
                nc.tensor.matmul(u_ps[:, h * S:(h + 1) * S],
                                 lhsT=wvT_sb[:, h, :], rhs=at4[:, h, :],
                                 start=True, stop=True)
            u_sb = small.tile([C, C], F32)
            nc.scalar.copy(out=u_sb, in_=u_ps)
            ut_ps = psS.tile([C, C], F32, tag="ph2")
            nc.tensor.transpose(ut_ps, u_sb, id32_sb)
            ut_sb = small.tile([C, C], F32)
            nc.scalar.copy(out=ut_sb, in_=ut_ps)
            w2_ps = psS.tile([C, C], F32, tag="ph2")
            nc.tensor.matmul(w2_ps, lhsT=ut_sb, rhs=wf_sb, start=True, stop=True)
            w2_sb = small.tile([C, C], F16)
            nc.vector.tensor_tensor(out=w2_sb, in0=w2_ps, in1=dg_sb, op=AOP.add)

            # ============ Phase 3: y'^T = W2'^T z'^T =======================
            ydt = F32  # non-transpose matmul PSUM output must be fp32
            YB = 4                       # chunks per output DMA
            ys = None
            for q in range(nychunk):
                yp = psY.tile([C, YCHUNK], ydt)
                zchunk = zT[:, q * SGRP:(q + 1) * SGRP].rearrange(
                    "c j t -> c (j t)")
                nc.tensor.matmul(yp, lhsT=w2_sb, rhs=zchunk,
                                 start=True, stop=True)
                if q % YB == 0:
                    ys = ybuf.tile([C, YB, YCHUNK], F16)
                if q % 2 == 0:
                    nc.vector.tensor_copy(out=ys[:, q % YB, :], in_=yp)
                else:
                    nc.scalar.copy(out=ys[:, q % YB, :], in_=yp)
                if q % YB == YB - 1:
                    q0 = q - (YB - 1)
                    nc.sync.dma_start(
                        out=yT_out[:, q0 * YCHUNK:(q + 1) * YCHUNK],
                        in_=ys)
    nc.compile()
    return nc


def _numpy_reference(x, gamma, beta, Wq, bq, Wk, bk, Wv, bv, Wf, bf, alpha):
    """Fallback for inputs outside the zero-bias fast path (never hit by the
    spec fills). Pure numpy replica of the jax reference."""
    Bx, Hx, Wx, Cx = x.shape
    t = Hx * Wx
    nh = NH
    s = Cx // nh
    xf = x.reshape(Bx, t, Cx).astype(np.float64)
    mu = xf.mean(-1, keepdims=True)
    var = ((xf - mu) ** 2).mean(-1, keepdims=True)
    xn = (xf - mu) / np.sqrt(var + EPS) * gamma + beta
    Q = (xn @ Wq + bq).reshape(Bx, t, nh, s)
    K = (xn @ Wk + bk).reshape(Bx, t, nh, s)
    V = (xn @ Wv + bv).reshape(Bx, t, nh, s)
    scores = np.einsum("bthi,bthj->bhij", K, Q) / float(alpha)
    scores = scores - scores.max(-1, keepdims=True)
    e = np.exp(scores)
    attn = e / e.sum(-1, keepdims=True)
    out = np.einsum("bthi,bhij->bthj", V, attn).reshape(Bx, t, Cx)
    y = out @ Wf + bf + xn
    return y.reshape(Bx, Hx, Wx, Cx).astype(np.float32)


_NC_CACHE = {}


def make_in_maps(inputs, tloc=TLOC, n_cores=N_CORES):
    x = np.asarray(inputs["x"], dtype=np.float32)
    gamma = np.asarray(inputs["gamma"], dtype=np.float32)
    Wq = np.asarray(inputs["Wq"], dtype=np.float32)
    Wk = np.asarray(inputs["Wk"], dtype=np.float32)
    Wv = np.asarray(inputs["Wv"], dtype=np.float32)
    Wf = np.asarray(inputs["Wf"], dtype=np.float32)
    sqC = np.sqrt(float(C)).astype(np.float32) if False else np.float32(np.sqrt(C))

    wq_g = np.ascontiguousarray(gamma[:, None] * Wq)
    wk_g = np.ascontiguousarray(gamma[:, None] * Wk)
    wv_g = gamma[:, None] * Wv
    wvT4 = np.ascontiguousarray(
        wv_g.T.reshape(NH, S, C).transpose(1, 0, 2).reshape(S, NH * C))
    wf_s = np.ascontiguousarray(Wf * sqC)
    dg_s = np.ascontiguousarray(np.diag(gamma).astype(np.float32) * sqC)
    ident_z = np.eye(P, dtype=np.float16)
    ident_f32 = np.eye(P, dtype=np.float32)

    nlg = tloc // (P * LGRP)
    x16 = x.astype(np.float16)
    xs = x16.reshape(n_cores, nlg, LGRP, P, C).transpose(0, 1, 3, 2, 4)
    xsp = np.zeros((n_cores, nlg, P, LGRP, CP), np.float16)
    xsp[..., :C] = xs
    xs = xsp.reshape(n_cores, nlg, P, LGRP * CP)
    shared = dict(wq_g=wq_g, wk_g=wk_g, wvT4=wvT4, wf_s=wf_s, dg_s=dg_s,
                  ident_z=ident_z, ident_f32=ident_f32)
    return [dict(shared, x_loc=xs[i]) for i in range(n_cores)]


def kernel(**inputs) -> np.ndarray:
    zero = lambda k: not np.any(np.asarray(inputs[k]))
    if not (zero("beta") and zero("bq") and zero("bk") and zero("bv")
            and zero("bf")):
        return _numpy_reference(**{k: np.asarray(v) for k, v in inputs.items()})

    inv_alpha = 1.0 / float(np.asarray(inputs["alpha"]))
    key = ("v2", TLOC, N_CORES, inv_alpha)
    if key not in _NC_CACHE:
        _NC_CACHE[key] = build_nc(TLOC, N_CORES, inv_alpha=inv_alpha)
    nc = _NC_CACHE[key]

    in_maps = make_in_maps(inputs)
    res = run_bass_kernel_spmd(nc, in_maps, core_ids=list(range(N_CORES)))
    yT = [res.results[i]["yT"] for i in range(N_CORES)]     # each [C, TLOC] f16
    y = np.concatenate([t.T.astype(np.float32) for t in yT], axis=0)
    return np.ascontiguousarray(y.reshape(B, HH, WW, C))


# revision 20
# speedup vs baseline: 1.0791x; 1.0791x over previous
"""MDTA (channel-attention transformer block) Trainium2 kernel, v2.

Math (validated against the jax reference):
  xn = LayerNorm(x) = z * gamma + beta,  z = (x - mu) * rsqrt(var + eps)
  Q/K/V = xn @ W* + b*;  scores_h = K_h^T Q_h / alpha  (per-head s x s, contracted
  over all t tokens);  attn = softmax(scores);  out = V @ blockdiag(attn)
  y = out @ Wf + bf + xn

With zero biases/beta (the spec fill), everything collapses to:
  G      = z^T z                        (C x C Gram, contracted over t)
  scores = Wk'^T G Wq' / alpha          (Wq' = diag(gamma) Wq, etc.)
  attn   = blockwise softmax(scores)    (4 diagonal 32x32 blocks)
  W2     = diag(g) Wv blockdiag(attn) Wf + diag(gamma)
  y      = z @ W2

Scaling trick: the device works with z' = z/sqrt(C) (rstd' = 1/sqrt(M2 + C*eps)
skips the /C in the variance). The C factor is restored in the score extraction
(x C) and by pre-scaling Wf and diag(gamma) by sqrt(C) on the host.

Sharding: 8 cores = (batch b in 0..3) x (token half in 0..1). Each core streams
its 32768-token slice in fp16: bn_stats (DVE) + even/odd merge -> mu, rstd';
z' = (x-mu)*rstd' split DVE(4x-mode)/ACT; Gram accumulation + transpose of z'
on PE with GPSIMD draining PSUM->SBUF. The last quarter of transposes is
deferred to overlap the pairwise AllReduce of G. Softmax/W2 are computed
redundantly per pair, then y'^T = W2'^T z'^T streams out in fp16 chunks.

v1 (previous session) measured 708us on HW: the per-element normalization sat
on GPSIMD at 2.26us/tile (7 G elem/s) and serialized the whole kernel. v2
moves it to DVE's 4x fp16 mode / ACT, halves DMA via fp16 I/O, drops bn_aggr
for a batched even/odd merge, and keeps PE near its ramped clock.
"""

import sys

import numpy as np

for _p in ("/opt/trn_rl_repo",):
    if _p not in sys.path:
        sys.path.append(_p)

import concourse.bacc as bacc
import concourse.bass as bass
import concourse.tile as tile
from concourse import mybir
from concourse.bass_utils import run_bass_kernel_spmd

B, HH, WW, C = 4, 256, 256, 128
NH, S = 4, 32
T = HH * WW            # tokens per batch
N_CORES = 8
TLOC = T // 2          # tokens per core
EPS = 1e-5
P = 128                # partitions / tile token count
LGRP = 8               # tiles per x-load DMA (2KB partition lines)
XB = 8                 # tiles per XBAR transpose instruction
SGRP = 4               # tiles per bn_stats call (free dim 512 = fmax)
MB = 8                 # stats-groups per merge batch (32 tiles)
XPAD = 0               # no pad: stats are per-tile accumulate ops
CP = C + XPAD
YCHUNK = SGRP * P      # output-stream chunk (1 PSUM bank)

F32 = mybir.dt.float32
F16 = mybir.dt.float16

AOP = mybir.AluOpType
AF = mybir.ActivationFunctionType


def build_nc(tloc=TLOC, n_cores=N_CORES, inv_alpha=1.0, zdt=F16, y_f32r=False,
             act_groups=5, defer_frac=0.25, y_psum_f16=True):
    """Build the SPMD Bass program. Cores 2b and 2b+1 hold the two token
    halves of batch b and pair up in the G all-reduce.

    act_groups: out of every 8 stats-groups, how many are normalized on ACT
    (the rest on DVE). defer_frac: fraction of transpose groups deferred to
    overlap the collective."""
    ntile = tloc // P
    nlg = ntile // LGRP
    nsg = ntile // SGRP
    mbs = min(MB, nsg)
    assert nsg % mbs == 0 and ntile % LGRP == 0
    nmb = nsg // mbs
    nychunk = tloc // YCHUNK            # == nsg
    dsg = max(1, int(round(nsg * (1.0 - defer_frac))))  # first deferred group
    sqC = float(np.sqrt(C))

    nc = bacc.Bacc("TRN2", target_bir_lowering=False, debug=False,
                   num_devices=n_cores)

    x_in = nc.declare_dram_parameter("x_loc", [nlg, P, LGRP * CP], F16,
                                     isOutput=False)   # host-repacked fp16
    wq_in = nc.declare_dram_parameter("wq_g", [C, C], F32, isOutput=False)
    wk_in = nc.declare_dram_parameter("wk_g", [C, C], F32, isOutput=False)
    wvT_in = nc.declare_dram_parameter("wvT4", [S, NH * C], F32, isOutput=False)
    wf_in = nc.declare_dram_parameter("wf_s", [C, C], F32, isOutput=False)   # sqrt(C)*Wf
    dg_in = nc.declare_dram_parameter("dg_s", [C, C], F32, isOutput=False)   # sqrt(C)*diag(gamma)
    idz_in = nc.declare_dram_parameter("ident_z", [P, P], F16, isOutput=False)
    id32_in = nc.declare_dram_parameter("ident_f32", [P, P], F32, isOutput=False)
    yT_out = nc.declare_dram_parameter("yT", [C, tloc], F16, isOutput=True)

    x_tiles = x_in.rearrange("g p (j c) -> g p j c", j=LGRP)  # c = CP
    replica_groups = [[2 * b, 2 * b + 1] for b in range(n_cores // 2)]

    # per-tile normalization engines, by measured rates
    # (DVE tensor_scalar 270ns, ACT activation 330ns, GPSIMD 2.26us)
    norm_eng = "avaavgavaaavgaavaavagaavaavaavav"  # 19a 10v 3g per 32 tiles

    with tile.TileContext(nc) as tc:
        with (
            tc.tile_pool(name="const", bufs=1) as const,
            tc.tile_pool(name="xload", bufs=10) as xload,
            tc.tile_pool(name="tmp", bufs=3) as tmp,
            tc.tile_pool(name="small", bufs=2) as small,
            tc.tile_pool(name="ybuf", bufs=4) as ybuf,
            tc.tile_pool(name="psA", bufs=1, space="PSUM") as psA,
            tc.tile_pool(name="psS", bufs=2, space="PSUM") as psS,
            tc.tile_pool(name="psY", bufs=4, space="PSUM") as psY,
            tc.tile_pool(name="dram", bufs=1, space="DRAM") as dram,
        ):
            # ---- constants ----
            wq_sb = const.tile([C, C], F32)
            wk_sb = const.tile([C, C], F32)
            wvT_sb = const.tile([S, NH, C], F32)
            wf_sb = const.tile([C, C], F32)
            dg_sb = const.tile([C, C], F32)
            idz_sb = const.tile([P, P], F16)
            id32_sb = const.tile([P, P], F32)
            nc.sync.dma_start(out=wq_sb, in_=wq_in[:])
            nc.sync.dma_start(out=wk_sb, in_=wk_in[:])
            nc.sync.dma_start(out=wvT_sb,
                              in_=wvT_in[:].rearrange("s (h c) -> s h c", h=NH))
            nc.sync.dma_start(out=wf_sb, in_=wf_in[:])
            nc.sync.dma_start(out=dg_sb, in_=dg_in[:])
            nc.sync.dma_start(out=idz_sb, in_=idz_in[:])
            nc.sync.dma_start(out=id32_sb, in_=id32_in[:])
            eps_sb = const.tile([P, 1], F32)
            nc.vector.memset(eps_sb, float(C) * EPS)

            # ---- big write-once state ----
            zbig = const.tile([P, ntile, C], F16)     # z' in [t, c]
            zT = const.tile([C, ntile, P], F16)       # z' transposed (flat [C, tloc])
            st6 = const.tile([P, nsg, SGRP, 8], F32)  # bn_stats out (6 used)
            mub = const.tile([P, nsg, SGRP], F32)     # mean
            stdb = const.tile([P, nsg, SGRP], F32)    # std' (ACT sqrt out)
            rsb = const.tile([P, nsg, SGRP], F32)     # rstd' = 1/sqrt(M2+C eps)
            nmb_t = const.tile([P, nsg, SGRP], F32)   # -mean*rstd' (ACT bias)
            G_ps = psA.tile([C, C], F32)              # Gram accumulator

            # ============ Phase 1: stats + norm + Gram (+ some T) ==========
            xh = [None] * nlg

            def do_xbar(xg, eng):
                """Transpose z' tiles [16*xg, 16*xg+16) via the DMA XBAR
                (SBUF->SBUF, per-128x128-tile): zT[c, i, t] = z[t, i, c]."""
                src_ap = zbig[:, XB * xg:XB * (xg + 1), :]
                eng.dma_start(out=zT[:, XB * xg:XB * (xg + 1)], in_=src_ap,
                              transpose=True)

            def load_and_stats(mb):
                nlg_b = mbs * SGRP // LGRP
                for lgi in range(nlg_b):
                    lg = mb * nlg_b + lgi
                    x8 = xload.tile([P, LGRP, CP], F16)
                    xh[lg] = x8
                    nc.sync.dma_start(out=x8, in_=x_tiles[lg])
                    for jl in range(LGRP):
                        i = lg * LGRP + jl
                        sg, jt = i // SGRP, i % SGRP
                        nc.vector.bn_stats(out=st6[:, sg, jt, 0:6],
                                           in_=x8[:, jl, 0:C])

            def merge_a(mb):
                """DVE moment combine + ACT sqrt for batch mb (no recip)."""
                g0, g1 = mbs * mb, mbs * mb + mbs
                me = st6[:, g0:g1, :, 1]
                mo = st6[:, g0:g1, :, 4]
                ve = st6[:, g0:g1, :, 2]
                vo = st6[:, g0:g1, :, 5]
                t_s = tmp.tile([P, mbs, SGRP], F32)
                t_d = tmp.tile([P, mbs, SGRP], F32)
                t_dd = tmp.tile([P, mbs, SGRP], F32)
                t_ve = tmp.tile([P, mbs, SGRP], F32)
                t_m2 = tmp.tile([P, mbs, SGRP], F32)
                t_std = stdb[:, g0:g1, :]
                mu = mub[:, g0:g1, :]
                nc.vector.tensor_tensor(out=t_s, in0=me, in1=mo, op=AOP.add)
                nc.vector.tensor_scalar_mul(out=mu, in0=t_s, scalar1=0.5)
                nc.vector.tensor_tensor(out=t_d, in0=me, in1=mo, op=AOP.subtract)
                nc.vector.tensor_tensor(out=t_dd, in0=t_d, in1=t_d, op=AOP.mult)
                nc.vector.tensor_tensor(out=t_ve, in0=ve, in1=vo, op=AOP.add)
                nc.vector.scalar_tensor_tensor(
                    out=t_m2, in0=t_dd, scalar=float(C) / 4.0, in1=t_ve,
                    op0=AOP.mult, op1=AOP.add)
                # std' = sqrt(M2 + C*eps) on ACT; latency hidden by the next
                # batch's bn_stats on DVE
                nc.scalar.activation(out=t_std, in_=t_m2, func=AF.Sqrt,
                                     bias=eps_sb[:], scale=1.0)

            def merge_b(mb):
                """rstd' = 1/std' and -mu*rstd' for batch mb."""
                g0, g1 = mbs * mb, mbs * mb + mbs
                t_mr = tmp.tile([P, mbs, SGRP], F32)
                mu = mub[:, g0:g1, :]
                rs = rsb[:, g0:g1, :]
                nm = nmb_t[:, g0:g1, :]
                nc.vector.reciprocal(out=rs, in_=stdb[:, g0:g1, :])
                nc.vector.tensor_tensor(out=t_mr, in0=mu, in1=rs, op=AOP.mult)
                nc.vector.tensor_scalar_mul(out=nm, in0=t_mr, scalar1=-1.0)

            def norm_gram(mb):
                for sgi in range(mbs):
                    sg = mbs * mb + sgi
                    for jt in range(SGRP):
                        i = sg * SGRP + jt
                        lg, jl = i // LGRP, i % LGRP
                        xs = xh[lg][:, jl, 0:C]
                        zi = zbig[:, i, :]
                        eng = norm_eng[i % len(norm_eng)]
                        if eng == "a":
                            nc.scalar.activation(
                                out=zi, in_=xs, func=AF.Identity,
                                bias=nmb_t[:, sg, jt:jt + 1],
                                scale=rsb[:, sg, jt:jt + 1])
                        elif eng == "g":
                            nc.gpsimd.tensor_scalar(
                                out=zi, in0=xs,
                                scalar1=mub[:, sg, jt:jt + 1],
                                scalar2=rsb[:, sg, jt:jt + 1],
                                op0=AOP.subtract, op1=AOP.mult)
                        else:
                            nc.vector.tensor_scalar(
                                out=zi, in0=xs,
                                scalar1=mub[:, sg, jt:jt + 1],
                                scalar2=rsb[:, sg, jt:jt + 1],
                                op0=AOP.subtract, op1=AOP.mult)
                        nc.tensor.matmul(G_ps, lhsT=zi, rhs=zi,
                                         start=(i == 0), stop=(i == ntile - 1))

            load_and_stats(0)
            merge_a(0)
            for mb in range(nmb):
                # merge_b first: ACT's norms unblock 3 small DVE ops in;
                # stats/merge_a for mb+1 then run under ACT's norm burst
                merge_b(mb)
                norm_gram(mb)
                if mb + 1 < nmb:
                    load_and_stats(mb + 1)
                    merge_a(mb + 1)
                if mb > 0:
                    for xg in range(mbs * SGRP * (mb - 1) // XB,
                                    mbs * SGRP * mb // XB):
                        do_xbar(xg, nc.sync)

            # ============ Phase 2: all-reduce G, softmax, W2 ===============
            for xg in range(mbs * SGRP * (nmb - 1) // XB, ntile // XB):
                do_xbar(xg, nc.sync)
            g_sb = small.tile([C, C], F32)
            nc.vector.tensor_copy(out=g_sb, in_=G_ps)
            g_in_d = dram.tile([C, C], F32)
            g_out_d = dram.tile([C, C], F32)
            nc.gpsimd.dma_start(out=g_in_d, in_=g_sb)
            nc.gpsimd.collective_compute(
                "AllReduce", AOP.add,
                replica_groups=replica_groups,
                ins=[g_in_d[:].opt()], outs=[g_out_d[:].opt()])

            gs_sb = small.tile([C, C], F32)
            nc.gpsimd.dma_start(out=gs_sb, in_=g_out_d)

            # scores = wk^T (G wq) * (C/alpha);  G symmetric so lhsT=G works
            s1_ps = psS.tile([C, C], F32, tag="ph2")
            nc.tensor.matmul(s1_ps, lhsT=gs_sb, rhs=wq_sb, start=True, stop=True)
            s1_sb = small.tile([C, C], F32)
            nc.scalar.copy(out=s1_sb, in_=s1_ps)
            sc_ps = psS.tile([C, C], F32, tag="ph2")
            nc.tensor.matmul(sc_ps, lhsT=wk_sb, rhs=s1_sb, start=True, stop=True)

            # extract 4 diagonal 32x32 blocks (x C/alpha) -> [128, 32]
            sm = small.tile([P, S], F32)
            for h in range(NH):
                nc.scalar.mul(out=sm[h * S:(h + 1) * S, :],
                              in_=sc_ps[h * S:(h + 1) * S, h * S:(h + 1) * S],
                              mul=float(C) * float(inv_alpha))
            mx = small.tile([P, 1], F32)
            nc.vector.reduce_max(mx, sm, mybir.AxisListType.X)
            nmx = small.tile([P, 1], F32)
            nc.vector.tensor_scalar_mul(out=nmx, in0=mx, scalar1=-1.0)
            sh = small.tile([P, S], F32)
            nc.vector.tensor_scalar(out=sh, in0=sm, scalar1=nmx, scalar2=-87.0,
                                    op0=AOP.add, op1=AOP.max)
            ex = small.tile([P, S], F32)
            es = small.tile([P, 1], F32)
            nc.scalar.activation(out=ex, in_=sh, func=AF.Exp,
                                 bias=0.0, scale=1.0, accum_out=es)
            ri = small.tile([P, 1], F32)
            nc.vector.reciprocal(out=ri, in_=es)
            at = small.tile([P, S], F32)
            nc.vector.tensor_scalar_mul(out=at, in0=ex, scalar1=ri)
            at4 = small.tile([S, NH, S], F32)
            for h in range(NH):
                nc.sync.dma_start(out=at4[:, h, :], in_=at[h * S:(h + 1) * S, :])

            # U = diag(g) Wv blockdiag(attn): per-head [128,32] matmuls
            u_ps = psS.tile([C, C], F32, tag="ph2")
            for h in range(NH):
                nc.tensor.matmul(u_ps[:, h * S:(h + 1) * S],
                                 lhsT=wvT_sb[:, h, :], rhs=at4[:, h, :],
                                 start=True, stop=True)
            u_sb = small.tile([C, C], F32)
            nc.scalar.copy(out=u_sb, in_=u_ps)
            ut_ps = psS.tile([C, C], F32, tag="ph2")
            nc.tensor.transpose(ut_ps, u_sb, id32_sb)
            ut_sb = small.tile([C, C], F32)
            nc.scalar.copy(out=ut_sb, in_=ut_ps)
            w2_ps = psS.tile([C, C], F32, tag="ph2")
            nc.tensor.matmul(w2_ps, lhsT=ut_sb, rhs=wf_sb, start=True, stop=True)
            w2_sb = small.tile([C, C], F16)
            nc.vector.tensor_tensor(out=w2_sb, in0=w2_ps, in1=dg_sb, op=AOP.add)

            # ============ Phase 3: y'^T = W2'^T z'^T =======================
            ydt = F32  # non-transpose matmul PSUM output must be fp32
            YB = 4                       # chunks per output DMA
            ys = None
            for q in range(nychunk):
                yp = psY.tile([C, YCHUNK], ydt)
                zchunk = zT[:, q * SGRP:(q + 1) * SGRP].rearrange(
                    "c j t -> c (j t)")
                nc.tensor.matmul(yp, lhsT=w2_sb, rhs=zchunk,
                                 start=True, stop=True)
                if q % YB == 0:
                    ys = ybuf.tile([C, YB, YCHUNK], F16)
                if q % 2 == 0:
                    nc.vector.tensor_copy(out=ys[:, q % YB, :], in_=yp)
                else:
                    nc.scalar.copy(out=ys[:, q % YB, :], in_=yp)
                if q % YB == YB - 1:
                    q0 = q - (YB - 1)
                    nc.sync.dma_start(
                        out=yT_out[:, q0 * YCHUNK:(q + 1) * YCHUNK],
                        in_=ys)
    nc.compile()
    return nc


def _numpy_reference(x, gamma, beta, Wq, bq, Wk, bk, Wv, bv, Wf, bf, alpha):
    """Fallback for inputs outside the zero-bias fast path (never hit by the
    spec fills). Pure numpy replica of the jax reference."""
    Bx, Hx, Wx, Cx = x.shape
    t = Hx * Wx
    nh = NH
    s = Cx // nh
    xf = x.reshape(Bx, t, Cx).astype(np.float64)
    mu = xf.mean(-1, keepdims=True)
    var = ((xf - mu) ** 2).mean(-1, keepdims=True)
    xn = (xf - mu) / np.sqrt(var + EPS) * gamma + beta
    Q = (xn @ Wq + bq).reshape(Bx, t, nh, s)
    K = (xn @ Wk + bk).reshape(Bx, t, nh, s)
    V = (xn @ Wv + bv).reshape(Bx, t, nh, s)
    scores = np.einsum("bthi,bthj->bhij", K, Q) / float(alpha)
    scores = scores - scores.max(-1, keepdims=True)
    e = np.exp(scores)
    attn = e / e.sum(-1, keepdims=True)
    out = np.einsum("bthi,bhij->bthj", V, attn).reshape(Bx, t, Cx)
    y = out @ Wf + bf + xn
    return y.reshape(Bx, Hx, Wx, Cx).astype(np.float32)


_NC_CACHE = {}


def make_in_maps(inputs, tloc=TLOC, n_cores=N_CORES):
    x = np.asarray(inputs["x"], dtype=np.float32)
    gamma = np.asarray(inputs["gamma"], dtype=np.float32)
    Wq = np.asarray(inputs["Wq"], dtype=np.float32)
    Wk = np.asarray(inputs["Wk"], dtype=np.float32)
    Wv = np.asarray(inputs["Wv"], dtype=np.float32)
    Wf = np.asarray(inputs["Wf"], dtype=np.float32)
    sqC = np.sqrt(float(C)).astype(np.float32) if False else np.float32(np.sqrt(C))

    wq_g = np.ascontiguousarray(gamma[:, None] * Wq)
    wk_g = np.ascontiguousarray(gamma[:, None] * Wk)
    wv_g = gamma[:, None] * Wv
    wvT4 = np.ascontiguousarray(
        wv_g.T.reshape(NH, S, C).transpose(1, 0, 2).reshape(S, NH * C))
    wf_s = np.ascontiguousarray(Wf * sqC)
    dg_s = np.ascontiguousarray(np.diag(gamma).astype(np.float32) * sqC)
    ident_z = np.eye(P, dtype=np.float16)
    ident_f32 = np.eye(P, dtype=np.float32)

    nlg = tloc // (P * LGRP)
    x16 = x.astype(np.float16)
    xs = x16.reshape(n_cores, nlg, LGRP, P, C).transpose(0, 1, 3, 2, 4)
    xsp = np.zeros((n_cores, nlg, P, LGRP, CP), np.float16)
    xsp[..., :C] = xs
    xs = xsp.reshape(n_cores, nlg, P, LGRP * CP)
    shared = dict(wq_g=wq_g, wk_g=wk_g, wvT4=wvT4, wf_s=wf_s, dg_s=dg_s,
                  ident_z=ident_z, ident_f32=ident_f32)
    return [dict(shared, x_loc=xs[i]) for i in range(n_cores)]


def kernel(**inputs) -> np.ndarray:
    zero = lambda k: not np.any(np.asarray(inputs[k]))
    if not (zero("beta") and zero("bq") and zero("bk") and zero("bv")
            and zero("bf")):
        return _numpy_reference(**{k: np.asarray(v) for k, v in inputs.items()})

    inv_alpha = 1.0 / float(np.asarray(inputs["alpha"]))
    key = ("v2", TLOC, N_CORES, inv_alpha)
    if key not in _NC_CACHE:
        _NC_CACHE[key] = build_nc(TLOC, N_CORES, inv_alpha=inv_alpha)
    nc = _NC_CACHE[key]

    in_maps = make_in_maps(inputs)
    res = run_bass_kernel_spmd(nc, in_maps, core_ids=list(range(N_CORES)))
    yT = [res.results[i]["yT"] for i in range(N_CORES)]     # each [C, TLOC] f16
    y = np.concatenate([t.T.astype(np.float32) for t in yT], axis=0)
    return np.ascontiguousarray(y.reshape(B, HH, WW, C))


# revision 21
# speedup vs baseline: 1.1159x; 1.0341x over previous
"""MDTA (channel-attention transformer block) Trainium2 kernel, v2.

Math (validated against the jax reference):
  xn = LayerNorm(x) = z * gamma + beta,  z = (x - mu) * rsqrt(var + eps)
  Q/K/V = xn @ W* + b*;  scores_h = K_h^T Q_h / alpha  (per-head s x s, contracted
  over all t tokens);  attn = softmax(scores);  out = V @ blockdiag(attn)
  y = out @ Wf + bf + xn

With zero biases/beta (the spec fill), everything collapses to:
  G      = z^T z                        (C x C Gram, contracted over t)
  scores = Wk'^T G Wq' / alpha          (Wq' = diag(gamma) Wq, etc.)
  attn   = blockwise softmax(scores)    (4 diagonal 32x32 blocks)
  W2     = diag(g) Wv blockdiag(attn) Wf + diag(gamma)
  y      = z @ W2

Scaling trick: the device works with z' = z/sqrt(C) (rstd' = 1/sqrt(M2 + C*eps)
skips the /C in the variance). The C factor is restored in the score extraction
(x C) and by pre-scaling Wf and diag(gamma) by sqrt(C) on the host.

Sharding: 8 cores = (batch b in 0..3) x (token half in 0..1). Each core streams
its 32768-token slice in fp16: bn_stats (DVE) + even/odd merge -> mu, rstd';
z' = (x-mu)*rstd' split DVE(4x-mode)/ACT; Gram accumulation + transpose of z'
on PE with GPSIMD draining PSUM->SBUF. The last quarter of transposes is
deferred to overlap the pairwise AllReduce of G. Softmax/W2 are computed
redundantly per pair, then y'^T = W2'^T z'^T streams out in fp16 chunks.

v1 (previous session) measured 708us on HW: the per-element normalization sat
on GPSIMD at 2.26us/tile (7 G elem/s) and serialized the whole kernel. v2
moves it to DVE's 4x fp16 mode / ACT, halves DMA via fp16 I/O, drops bn_aggr
for a batched even/odd merge, and keeps PE near its ramped clock.
"""

import sys

import numpy as np

for _p in ("/opt/trn_rl_repo",):
    if _p not in sys.path:
        sys.path.append(_p)

import concourse.bacc as bacc
import concourse.bass as bass
import concourse.tile as tile
from concourse import mybir
from concourse.bass_utils import run_bass_kernel_spmd

B, HH, WW, C = 4, 256, 256, 128
NH, S = 4, 32
T = HH * WW            # tokens per batch
N_CORES = 8
TLOC = T // 2          # tokens per core
EPS = 1e-5
P = 128                # partitions / tile token count
LGRP = 8               # tiles per x-load DMA (2KB partition lines)
XB = 8                 # tiles per XBAR transpose instruction
SGRP = 4               # tiles per bn_stats call (free dim 512 = fmax)
MB = 8                 # stats-groups per merge batch (32 tiles)
XPAD = 0               # no pad: stats are per-tile accumulate ops
CP = C + XPAD
YCHUNK = SGRP * P      # output-stream chunk (1 PSUM bank)

F32 = mybir.dt.float32
F16 = mybir.dt.float16

AOP = mybir.AluOpType
AF = mybir.ActivationFunctionType


def build_nc(tloc=TLOC, n_cores=N_CORES, inv_alpha=1.0, zdt=F16, y_f32r=False,
             act_groups=5, defer_frac=0.25, y_psum_f16=True):
    """Build the SPMD Bass program. Cores 2b and 2b+1 hold the two token
    halves of batch b and pair up in the G all-reduce.

    act_groups: out of every 8 stats-groups, how many are normalized on ACT
    (the rest on DVE). defer_frac: fraction of transpose groups deferred to
    overlap the collective."""
    ntile = tloc // P
    nlg = ntile // LGRP
    nsg = ntile // SGRP
    mbs = min(MB, nsg)
    assert nsg % mbs == 0 and ntile % LGRP == 0
    nmb = nsg // mbs
    nychunk = tloc // YCHUNK            # == nsg
    dsg = max(1, int(round(nsg * (1.0 - defer_frac))))  # first deferred group
    sqC = float(np.sqrt(C))

    nc = bacc.Bacc("TRN2", target_bir_lowering=False, debug=False,
                   num_devices=n_cores)

    x_in = nc.declare_dram_parameter("x_loc", [nlg, P, LGRP * CP], F16,
                                     isOutput=False)   # host-repacked fp16
    wq_in = nc.declare_dram_parameter("wq_g", [C, C], F32, isOutput=False)
    wk_in = nc.declare_dram_parameter("wk_g", [C, C], F32, isOutput=False)
    wvT_in = nc.declare_dram_parameter("wvT4", [S, NH * C], F32, isOutput=False)
    wf_in = nc.declare_dram_parameter("wf_s", [C, C], F32, isOutput=False)   # sqrt(C)*Wf
    dg_in = nc.declare_dram_parameter("dg_s", [C, C], F32, isOutput=False)   # sqrt(C)*diag(gamma)
    idz_in = nc.declare_dram_parameter("ident_z", [P, P], F16, isOutput=False)
    id32_in = nc.declare_dram_parameter("ident_f32", [P, P], F32, isOutput=False)
    yT_out = nc.declare_dram_parameter("yT", [C, tloc], F16, isOutput=True)

    x_tiles = x_in.rearrange("g p (j c) -> g p j c", j=LGRP)  # c = CP
    replica_groups = [[2 * b, 2 * b + 1] for b in range(n_cores // 2)]

    # per-tile normalization engines, by measured rates
    # (DVE tensor_scalar 270ns, ACT activation 330ns, GPSIMD 2.26us)
    norm_eng = "gavaavaavaagvaavaavaagvaavaavaav"  # 19a 10v 3g, GP early

    with tile.TileContext(nc) as tc:
        with (
            tc.tile_pool(name="const", bufs=1) as const,
            tc.tile_pool(name="xload", bufs=10) as xload,
            tc.tile_pool(name="small", bufs=2) as small,
            tc.tile_pool(name="ybuf", bufs=4) as ybuf,
            tc.tile_pool(name="psA", bufs=1, space="PSUM") as psA,
            tc.tile_pool(name="psS", bufs=2, space="PSUM") as psS,
            tc.tile_pool(name="psY", bufs=4, space="PSUM") as psY,
            tc.tile_pool(name="dram", bufs=1, space="DRAM") as dram,
        ):
            # ---- constants ----
            wq_sb = const.tile([C, C], F32)
            wk_sb = const.tile([C, C], F32)
            wvT_sb = const.tile([S, NH, C], F32)
            wf_sb = const.tile([C, C], F32)
            dg_sb = const.tile([C, C], F32)
            idz_sb = const.tile([P, P], F16)
            id32_sb = const.tile([P, P], F32)
            nc.sync.dma_start(out=wq_sb, in_=wq_in[:])
            nc.sync.dma_start(out=wk_sb, in_=wk_in[:])
            nc.sync.dma_start(out=wvT_sb,
                              in_=wvT_in[:].rearrange("s (h c) -> s h c", h=NH))
            nc.sync.dma_start(out=wf_sb, in_=wf_in[:])
            nc.sync.dma_start(out=dg_sb, in_=dg_in[:])
            nc.sync.dma_start(out=idz_sb, in_=idz_in[:])
            nc.sync.dma_start(out=id32_sb, in_=id32_in[:])
            eps_sb = const.tile([P, 1], F32)
            nc.vector.memset(eps_sb, float(C) * EPS)

            # ---- big write-once state ----
            zbig = const.tile([P, ntile, C], F16)     # z' in [t, c]
            zT = const.tile([C, ntile, P], F16)       # z' transposed (flat [C, tloc])
            st6 = const.tile([P, nsg, SGRP, 8], F32)  # bn_stats out (6 used)
            mub = const.tile([P, nsg, SGRP], F32)     # mean
            stdb = const.tile([P, nsg, SGRP], F32)    # std' (ACT sqrt out)
            t_sb = const.tile([P, nsg, SGRP], F32)
            t_db = const.tile([P, nsg, SGRP], F32)
            t_ddb = const.tile([P, nsg, SGRP], F32)
            t_veb = const.tile([P, nsg, SGRP], F32)
            t_m2b = const.tile([P, nsg, SGRP], F32)
            t_mrb = const.tile([P, nsg, SGRP], F32)
            rsb = const.tile([P, nsg, SGRP], F32)     # rstd' = 1/sqrt(M2+C eps)
            nmb_t = const.tile([P, nsg, SGRP], F32)   # -mean*rstd' (ACT bias)
            G_ps = psA.tile([C, C], F32)              # Gram accumulator

            # ============ Phase 1: stats + norm + Gram (+ some T) ==========
            xh = [None] * nlg

            def do_xbar(xg, eng):
                """Transpose z' tiles [16*xg, 16*xg+16) via the DMA XBAR
                (SBUF->SBUF, per-128x128-tile): zT[c, i, t] = z[t, i, c]."""
                src_ap = zbig[:, XB * xg:XB * (xg + 1), :]
                eng.dma_start(out=zT[:, XB * xg:XB * (xg + 1)], in_=src_ap,
                              transpose=True)

            def load_and_stats(mb):
                nlg_b = mbs * SGRP // LGRP
                for lgi in range(nlg_b):
                    lg = mb * nlg_b + lgi
                    x8 = xload.tile([P, LGRP, CP], F16)
                    xh[lg] = x8
                    nc.sync.dma_start(out=x8, in_=x_tiles[lg])
                    for jl in range(LGRP):
                        i = lg * LGRP + jl
                        sg, jt = i // SGRP, i % SGRP
                        nc.vector.bn_stats(out=st6[:, sg, jt, 0:6],
                                           in_=x8[:, jl, 0:C])

            def merge_a(mb):
                """DVE moment combine + ACT sqrt for batch mb (no recip)."""
                g0, g1 = mbs * mb, mbs * mb + mbs
                me = st6[:, g0:g1, :, 1]
                mo = st6[:, g0:g1, :, 4]
                ve = st6[:, g0:g1, :, 2]
                vo = st6[:, g0:g1, :, 5]
                t_s = t_sb[:, g0:g1, :]
                t_d = t_db[:, g0:g1, :]
                t_dd = t_ddb[:, g0:g1, :]
                t_ve = t_veb[:, g0:g1, :]
                t_m2 = t_m2b[:, g0:g1, :]
                t_std = stdb[:, g0:g1, :]
                mu = mub[:, g0:g1, :]
                nc.vector.tensor_tensor(out=t_s, in0=me, in1=mo, op=AOP.add)
                nc.vector.tensor_scalar_mul(out=mu, in0=t_s, scalar1=0.5)
                nc.vector.tensor_tensor(out=t_d, in0=me, in1=mo, op=AOP.subtract)
                nc.vector.tensor_tensor(out=t_dd, in0=t_d, in1=t_d, op=AOP.mult)
                nc.vector.tensor_tensor(out=t_ve, in0=ve, in1=vo, op=AOP.add)
                nc.vector.scalar_tensor_tensor(
                    out=t_m2, in0=t_dd, scalar=float(C) / 4.0, in1=t_ve,
                    op0=AOP.mult, op1=AOP.add)
                # std' = sqrt(M2 + C*eps) on ACT; latency hidden by the next
                # batch's bn_stats on DVE
                nc.scalar.activation(out=t_std, in_=t_m2, func=AF.Sqrt,
                                     bias=eps_sb[:], scale=1.0)

            def merge_b(mb):
                """rstd' = 1/std' and -mu*rstd' for batch mb."""
                g0, g1 = mbs * mb, mbs * mb + mbs
                t_mr = t_mrb[:, g0:g1, :]
                mu = mub[:, g0:g1, :]
                rs = rsb[:, g0:g1, :]
                nm = nmb_t[:, g0:g1, :]
                nc.vector.reciprocal(out=rs, in_=stdb[:, g0:g1, :])
                nc.vector.tensor_tensor(out=t_mr, in0=mu, in1=rs, op=AOP.mult)
                nc.vector.tensor_scalar_mul(out=nm, in0=t_mr, scalar1=-1.0)

            def norm_gram(mb):
                for sgi in range(mbs):
                    sg = mbs * mb + sgi
                    for jt in range(SGRP):
                        i = sg * SGRP + jt
                        lg, jl = i // LGRP, i % LGRP
                        xs = xh[lg][:, jl, 0:C]
                        zi = zbig[:, i, :]
                        eng = norm_eng[i % len(norm_eng)]
                        if eng == "a":
                            nc.scalar.activation(
                                out=zi, in_=xs, func=AF.Identity,
                                bias=nmb_t[:, sg, jt:jt + 1],
                                scale=rsb[:, sg, jt:jt + 1])
                        elif eng == "g":
                            nc.gpsimd.tensor_scalar(
                                out=zi, in0=xs,
                                scalar1=mub[:, sg, jt:jt + 1],
                                scalar2=rsb[:, sg, jt:jt + 1],
                                op0=AOP.subtract, op1=AOP.mult)
                        else:
                            nc.vector.tensor_scalar(
                                out=zi, in0=xs,
                                scalar1=mub[:, sg, jt:jt + 1],
                                scalar2=rsb[:, sg, jt:jt + 1],
                                op0=AOP.subtract, op1=AOP.mult)
                        nc.tensor.matmul(G_ps, lhsT=zi, rhs=zi,
                                         start=(i == 0), stop=(i == ntile - 1))

            load_and_stats(0)
            merge_a(0)
            for mb in range(nmb):
                # merge_b first: ACT's norms unblock 3 small DVE ops in;
                # stats/merge_a for mb+1 then run under ACT's norm burst
                merge_b(mb)
                norm_gram(mb)
                if mb + 1 < nmb:
                    load_and_stats(mb + 1)
                    merge_a(mb + 1)
                if mb > 0:
                    for xg in range(mbs * SGRP * (mb - 1) // XB,
                                    mbs * SGRP * mb // XB):
                        do_xbar(xg, nc.sync)

            # ============ Phase 2: all-reduce G, softmax, W2 ===============
            for xg in range(mbs * SGRP * (nmb - 1) // XB, ntile // XB):
                do_xbar(xg, nc.sync)
            g_sb = small.tile([C, C], F32)
            nc.vector.tensor_copy(out=g_sb, in_=G_ps)
            g_in_d = dram.tile([C, C], F32)
            g_out_d = dram.tile([C, C], F32)
            nc.gpsimd.dma_start(out=g_in_d, in_=g_sb)
            nc.gpsimd.collective_compute(
                "AllReduce", AOP.add,
                replica_groups=replica_groups,
                ins=[g_in_d[:].opt()], outs=[g_out_d[:].opt()])

            gs_sb = small.tile([C, C], F32)
            nc.gpsimd.dma_start(out=gs_sb, in_=g_out_d)

            # scores = wk^T (G wq) * (C/alpha);  G symmetric so lhsT=G works
            s1_ps = psS.tile([C, C], F32, tag="ph2")
            nc.tensor.matmul(s1_ps, lhsT=gs_sb, rhs=wq_sb, start=True, stop=True)
            s1_sb = small.tile([C, C], F32)
            nc.scalar.copy(out=s1_sb, in_=s1_ps)
            sc_ps = psS.tile([C, C], F32, tag="ph2")
            nc.tensor.matmul(sc_ps, lhsT=wk_sb, rhs=s1_sb, start=True, stop=True)

            # extract 4 diagonal 32x32 blocks (x C/alpha) -> [128, 32]
            sm = small.tile([P, S], F32)
            for h in range(NH):
                nc.scalar.mul(out=sm[h * S:(h + 1) * S, :],
                              in_=sc_ps[h * S:(h + 1) * S, h * S:(h + 1) * S],
                              mul=float(C) * float(inv_alpha))
            mx = small.tile([P, 1], F32)
            nc.vector.reduce_max(mx, sm, mybir.AxisListType.X)
            nmx = small.tile([P, 1], F32)
            nc.vector.tensor_scalar_mul(out=nmx, in0=mx, scalar1=-1.0)
            sh = small.tile([P, S], F32)
            nc.vector.tensor_scalar(out=sh, in0=sm, scalar1=nmx, scalar2=-87.0,
                                    op0=AOP.add, op1=AOP.max)
            ex = small.tile([P, S], F32)
            es = small.tile([P, 1], F32)
            nc.scalar.activation(out=ex, in_=sh, func=AF.Exp,
                                 bias=0.0, scale=1.0, accum_out=es)
            ri = small.tile([P, 1], F32)
            nc.vector.reciprocal(out=ri, in_=es)
            at = small.tile([P, S], F32)
            nc.vector.tensor_scalar_mul(out=at, in0=ex, scalar1=ri)
            at4 = small.tile([S, NH, S], F32)
            for h in range(NH):
                nc.sync.dma_start(out=at4[:, h, :], in_=at[h * S:(h + 1) * S, :])

            # U = diag(g) Wv blockdiag(attn): per-head [128,32] matmuls
            u_ps = psS.tile([C, C], F32, tag="ph2")
            for h in range(NH):
                nc.tensor.matmul(u_ps[:, h * S:(h + 1) * S],
                                 lhsT=wvT_sb[:, h, :], rhs=at4[:, h, :],
                                 start=True, stop=True)
            u_sb = small.tile([C, C], F32)
            nc.scalar.copy(out=u_sb, in_=u_ps)
            ut_ps = psS.tile([C, C], F32, tag="ph2")
            nc.tensor.transpose(ut_ps, u_sb, id32_sb)
            ut_sb = small.tile([C, C], F32)
            nc.scalar.copy(out=ut_sb, in_=ut_ps)
            w2_ps = psS.tile([C, C], F32, tag="ph2")
            nc.tensor.matmul(w2_ps, lhsT=ut_sb, rhs=wf_sb, start=True, stop=True)
            w2_sb = small.tile([C, C], F16)
            nc.vector.tensor_tensor(out=w2_sb, in0=w2_ps, in1=dg_sb, op=AOP.add)

            # ============ Phase 3: y'^T = W2'^T z'^T =======================
            ydt = F32  # non-transpose matmul PSUM output must be fp32
            YB = 4                       # chunks per output DMA
            ys = None
            for q in range(nychunk):
                yp = psY.tile([C, YCHUNK], ydt)
                zchunk = zT[:, q * SGRP:(q + 1) * SGRP].rearrange(
                    "c j t -> c (j t)")
                nc.tensor.matmul(yp, lhsT=w2_sb, rhs=zchunk,
                                 start=True, stop=True)
                if q % YB == 0:
                    ys = ybuf.tile([C, YB, YCHUNK], F16)
                if q % 2 == 0:
                    nc.vector.tensor_copy(out=ys[:, q % YB, :], in_=yp)
                else:
                    nc.scalar.copy(out=ys[:, q % YB, :], in_=yp)
                if q % YB == YB - 1:
                    q0 = q - (YB - 1)
                    nc.sync.dma_start(
                        out=yT_out[:, q0 * YCHUNK:(q + 1) * YCHUNK],
                        in_=ys)
    nc.compile()
    return nc


def _numpy_reference(x, gamma, beta, Wq, bq, Wk, bk, Wv, bv, Wf, bf, alpha):
    """Fallback for inputs outside the zero-bias fast path (never hit by the
    spec fills). Pure numpy replica of the jax reference."""
    Bx, Hx, Wx, Cx = x.shape
    t = Hx * Wx
    nh = NH
    s = Cx // nh
    xf = x.reshape(Bx, t, Cx).astype(np.float64)
    mu = xf.mean(-1, keepdims=True)
    var = ((xf - mu) ** 2).mean(-1, keepdims=True)
    xn = (xf - mu) / np.sqrt(var + EPS) * gamma + beta
    Q = (xn @ Wq + bq).reshape(Bx, t, nh, s)
    K = (xn @ Wk + bk).reshape(Bx, t, nh, s)
    V = (xn @ Wv + bv).reshape(Bx, t, nh, s)
    scores = np.einsum("bthi,bthj->bhij", K, Q) / float(alpha)
    scores = scores - scores.max(-1, keepdims=True)
    e = np.exp(scores)
    attn = e / e.sum(-1, keepdims=True)
    out = np.einsum("bthi,bhij->bthj", V, attn).reshape(Bx, t, Cx)
    y = out @ Wf + bf + xn
    return y.reshape(Bx, Hx, Wx, Cx).astype(np.float32)


_NC_CACHE = {}


def make_in_maps(inputs, tloc=TLOC, n_cores=N_CORES):
    x = np.asarray(inputs["x"], dtype=np.float32)
    gamma = np.asarray(inputs["gamma"], dtype=np.float32)
    Wq = np.asarray(inputs["Wq"], dtype=np.float32)
    Wk = np.asarray(inputs["Wk"], dtype=np.float32)
    Wv = np.asarray(inputs["Wv"], dtype=np.float32)
    Wf = np.asarray(inputs["Wf"], dtype=np.float32)
    sqC = np.sqrt(float(C)).astype(np.float32) if False else np.float32(np.sqrt(C))

    wq_g = np.ascontiguousarray(gamma[:, None] * Wq)
    wk_g = np.ascontiguousarray(gamma[:, None] * Wk)
    wv_g = gamma[:, None] * Wv
    wvT4 = np.ascontiguousarray(
        wv_g.T.reshape(NH, S, C).transpose(1, 0, 2).reshape(S, NH * C))
    wf_s = np.ascontiguousarray(Wf * sqC)
    dg_s = np.ascontiguousarray(np.diag(gamma).astype(np.float32) * sqC)
    ident_z = np.eye(P, dtype=np.float16)
    ident_f32 = np.eye(P, dtype=np.float32)

    nlg = tloc // (P * LGRP)
    x16 = x.astype(np.float16)
    xs = x16.reshape(n_cores, nlg, LGRP, P, C).transpose(0, 1, 3, 2, 4)
    xsp = np.zeros((n_cores, nlg, P, LGRP, CP), np.float16)
    xsp[..., :C] = xs
    xs = xsp.reshape(n_cores, nlg, P, LGRP * CP)
    shared = dict(wq_g=wq_g, wk_g=wk_g, wvT4=wvT4, wf_s=wf_s, dg_s=dg_s,
                  ident_z=ident_z, ident_f32=ident_f32)
    return [dict(shared, x_loc=xs[i]) for i in range(n_cores)]


def kernel(**inputs) -> np.ndarray:
    zero = lambda k: not np.any(np.asarray(inputs[k]))
    if not (zero("beta") and zero("bq") and zero("bk") and zero("bv")
            and zero("bf")):
        return _numpy_reference(**{k: np.asarray(v) for k, v in inputs.items()})

    inv_alpha = 1.0 / float(np.asarray(inputs["alpha"]))
    key = ("v2", TLOC, N_CORES, inv_alpha)
    if key not in _NC_CACHE:
        _NC_CACHE[key] = build_nc(TLOC, N_CORES, inv_alpha=inv_alpha)
    nc = _NC_CACHE[key]

    in_maps = make_in_maps(inputs)
    res = run_bass_kernel_spmd(nc, in_maps, core_ids=list(range(N_CORES)))
    yT = [res.results[i]["yT"] for i in range(N_CORES)]     # each [C, TLOC] f16
    y = np.concatenate([t.T.astype(np.float32) for t in yT], axis=0)
    return np.ascontiguousarray(y.reshape(B, HH, WW, C))
